# revision 1
# baseline (speedup 1.0000x reference)
"""Trainium2 Bass kernel for ConditionalFilterLayer.

Reference computation (per sample b):
  aux   = sigmoid(mask_w @ x + mask_b)          [K, HW]
  cf    = (aux @ x.T) / HW                      [K, C]
  filt  = batched_k(filt_w[k] @ cf[k]) + filt_b [K, C]
  pred  = filt @ x                              [K, HW]

Sharding: data-parallel over batch (B=8 == 8 cores, one sample per core),
small weights replicated. All matmuls in bf16 (fp32 PSUM accumulation).

Device layout choices (per core):
  x_nat  bf16 [4, 128, 16384]  c-major (c = 128j + p), kept resident in SBUF
  xT     bf16 [16, 128, 4096]  hw-major chunks for the pooling contraction
  fw     bf16 [32, 128, 4096]  filt_w[k].T packed per k-pair for wide DMA
Outputs are written bf16 and upcast to fp32 on the host.
The pooling contraction (over hw) uses PE-transposed aux chunks as lhsT and
the host-pretransposed xT as rhs, accumulating [K, C] in one PSUM bank.
The per-k filter matvec streams fw once through the PE with a masked cf
column as stationary, accumulating all K rows in one PSUM bank.
"""
import sys

if "/opt/trn_rl_repo" not in sys.path:
    sys.path.insert(0, "/opt/trn_rl_repo")

import numpy as np
import ml_dtypes

import concourse.bass as bass
import concourse.mybir as mybir
import concourse.tile as tile
from concourse import bacc
from concourse.bass_utils import run_bass_kernel_spmd
from concourse.masks import make_identity

BF16 = mybir.dt.bfloat16
F32 = mybir.dt.float32

B, C, K, H, W = 8, 512, 64, 128, 128
HW = H * W            # 16384
P = 128
CJ = C // P           # 4 contraction chunks
NCH = HW // 512       # 32 hw chunks of 512
N_CORES = 8

_NC_CACHE = {}

# tuning knobs (overridable for experiments)
CFG = dict(
    fw_bufs=4,      # phase-B weight stream double/triple buffering
    fw_kg=2,        # k's per fw DMA tile
    out_grp=4,      # output chunks batched per DMA
    xt_bufs=2,
    psA_bufs=4,
    psT_bufs=2,
)


def build_nc(iters: int = 1, **over):
    """Build + compile the per-core Bass kernel. Cached per (iters, cfg)."""
    cfg = {**CFG, **over}
    key = (iters, tuple(sorted(cfg.items())))
    if key in _NC_CACHE:
        return _NC_CACHE[key]
    fw_kg = cfg["fw_kg"]
    out_grp = cfg["out_grp"]

    nc = bacc.Bacc("TRN2", target_bir_lowering=False, debug=False)

    x_dram = nc.dram_tensor("x_nat", [CJ, P, HW], BF16, kind="ExternalInput")
    xt_dram = nc.dram_tensor("xT", [NCH // 2, P, 4096], BF16, kind="ExternalInput")
    mw_dram = nc.dram_tensor("mask_wT", [CJ, P, K], BF16, kind="ExternalInput")
    mb_dram = nc.dram_tensor("mask_b", [K, 1], F32, kind="ExternalInput")
    fw_dram = nc.dram_tensor("fw", [K // fw_kg, P, fw_kg * CJ * 512], BF16,
                             kind="ExternalInput")
    fb_dram = nc.dram_tensor("filt_b", [K, 512], F32, kind="ExternalInput")
    aux_dram = nc.dram_tensor("aux", [K, HW], BF16, kind="ExternalOutput")
    pred_dram = nc.dram_tensor("pred", [K, HW], BF16, kind="ExternalOutput")

    with tile.TileContext(nc) as tc:
        with (
            tc.tile_pool(name="const", bufs=1) as constp,
            tc.tile_pool(name="xres", bufs=1) as xresp,
            tc.tile_pool(name="xt", bufs=cfg["xt_bufs"]) as xtp,
            tc.tile_pool(name="fw", bufs=cfg["fw_bufs"]) as fwp,
            tc.tile_pool(name="auxf", bufs=2) as auxfp,
            tc.tile_pool(name="auxT", bufs=3) as auxTp,
            tc.tile_pool(name="small", bufs=1) as smallp,
            tc.tile_pool(name="predf", bufs=2) as predfp,
            tc.tile_pool(name="psA", bufs=cfg["psA_bufs"], space="PSUM") as psA,
            tc.tile_pool(name="psT", bufs=cfg["psT_bufs"], space="PSUM") as psT,
            tc.tile_pool(name="psCF", bufs=1, space="PSUM") as psCF,
        ):
            # --- constants (outside the timing loop) ---
            ident = constp.tile([P, P], BF16)
            make_identity(nc, ident[:])
            mw_sb = constp.tile([P, CJ, K], BF16)
            nc.sync.dma_start(mw_sb[:], mw_dram.rearrange("j p k -> p j k"))
            mb_sb = constp.tile([K, 1], F32)
            nc.sync.dma_start(mb_sb[:], mb_dram[:])
            fb_sb = constp.tile([K, 512], F32)
            nc.sync.dma_start(fb_sb[:], fb_dram[:])
            # rotating masked-cf stationary tiles (zeroed once; the per-k
            # writer also re-zeroes the column it wrote 8 k's ago)
            cfTm = constp.tile([P, 8, CJ, K], BF16)
            nc.gpsimd.memset(cfTm[:], 0.0)

            def body(_iv):
                # resident x (bf16, 128 KB/partition)
                x_res = xresp.tile([P, CJ, HW], BF16)
                for j in range(CJ):
                    for h in range(4):
                        nc.sync.dma_start(
                            x_res[:, j, h * 4096:(h + 1) * 4096],
                            x_dram[j, :, h * 4096:(h + 1) * 4096],
                        )

                # ---------- phase A: aux = sigmoid(mask_w @ x); cf = aux @ x.T
                ps_cf = psCF.tile([K, 512], F32)
                for i in range(NCH):
                    ps_logit = psA.tile([K, 512], F32, tag="psA")
                    for j in range(CJ):
                        nc.tensor.matmul(
                            ps_logit[:], mw_sb[:, j, :],
                            x_res[:, j, i * 512:(i + 1) * 512],
                            start=(j == 0), stop=(j == CJ - 1),
                        )
                    if i % out_grp == 0:
                        aux_g = auxfp.tile([K, out_grp, 512], BF16)
                    aux_b = aux_g[:, i % out_grp, :]
                    nc.scalar.activation(
                        aux_b, ps_logit[:],
                        mybir.ActivationFunctionType.Sigmoid,
                        bias=mb_sb[:, 0:1],
                    )
                    if i % out_grp == out_grp - 1:
                        nc.sync.dma_start(
                            aux_dram[:, (i - out_grp + 1) * 512:(i + 1) * 512],
                            aux_g[:],
                        )

                    if i % 2 == 0:
                        xt_t = xtp.tile([P, 2, CJ, 512], BF16)
                        nc.sync.dma_start(xt_t[:], xt_dram[i // 2])
                    auxT_t = auxTp.tile([P, CJ, K], BF16)
                    ps_tr = psT.tile([P, CJ, K], BF16, tag="psT")
                    for t in range(4):
                        nc.tensor.transpose(
                            ps_tr[:, t, :], aux_b[:, t * 128:(t + 1) * 128],
                            ident[:K, :K],
                        )
                    nc.vector.tensor_copy(auxT_t[:], ps_tr[:])
                    for t in range(4):
                        nc.tensor.matmul(
                            ps_cf[:], auxT_t[:, t, :],
                            xt_t[:, i % 2, t, :],
                            start=(i == 0 and t == 0),
                            stop=(i == NCH - 1 and t == 3),
                        )

                # ---------- phase B: filt[k] = filt_w[k] @ (cf[k]/HW) + filt_b
                cf_bf = smallp.tile([K, 512], BF16, tag="cf_bf")
                nc.scalar.activation(
                    cf_bf[:], ps_cf[:], mybir.ActivationFunctionType.Copy,
                    scale=1.0 / HW,
                )
                cfT = smallp.tile([P, CJ, K], BF16, tag="cfT")
                ps_tr = psT.tile([P, CJ, K], BF16, tag="psT")
                for t in range(4):
                    nc.tensor.transpose(
                        ps_tr[:, t, :], cf_bf[:, t * 128:(t + 1) * 128],
                        ident[:K, :K],
                    )
                nc.vector.tensor_copy(cfT[:], ps_tr[:])

                ps_filt = psA.tile([K, 512], F32, tag="psA")
                for g in range(K // fw_kg):
                    fw_t = fwp.tile([P, fw_kg * CJ * 512], BF16)
                    nc.sync.dma_start(fw_t[:], fw_dram[g])
                    for kk in range(fw_kg):
                        k = fw_kg * g + kk
                        r = k % 8
                        if k >= 8:
                            nc.vector.memset(cfTm[:, r, :, k - 8], 0.0)
                        nc.vector.tensor_copy(cfTm[:, r, :, k], cfT[:, :, k])
                        for j in range(CJ):
                            nc.tensor.matmul(
                                ps_filt[:], cfTm[:, r, j, :],
                                fw_t[:, (kk * CJ + j) * 512:(kk * CJ + j + 1) * 512],
                                start=(k == 0 and j == 0),
                                stop=(k == K - 1 and j == CJ - 1),
                            )

                filt_bf = smallp.tile([K, 512], BF16, tag="filt_bf")
                nc.vector.tensor_add(filt_bf[:], ps_filt[:], fb_sb[:])
                filtT = smallp.tile([P, CJ, K], BF16, tag="filtT")
                ps_tr = psT.tile([P, CJ, K], BF16, tag="psT")
                for t in range(4):
                    nc.tensor.transpose(
                        ps_tr[:, t, :], filt_bf[:, t * 128:(t + 1) * 128],
                        ident[:K, :K],
                    )
                nc.vector.tensor_copy(filtT[:], ps_tr[:])

                # ---------- phase C: pred = filt @ x
                for i in range(NCH):
                    ps_pred = psA.tile([K, 512], F32, tag="psA")
                    for j in range(CJ):
                        nc.tensor.matmul(
                            ps_pred[:], filtT[:, j, :],
                            x_res[:, j, i * 512:(i + 1) * 512],
                            start=(j == 0), stop=(j == CJ - 1),
                        )
                    if i % out_grp == 0:
                        pred_g = predfp.tile([K, out_grp, 512], BF16)
                    nc.vector.tensor_copy(pred_g[:, i % out_grp, :], ps_pred[:])
                    if i % out_grp == out_grp - 1:
                        nc.sync.dma_start(
                            pred_dram[:, (i - out_grp + 1) * 512:(i + 1) * 512],
                            pred_g[:],
                        )

            if iters == 1:
                body(None)
            else:
                with tc.For_i(0, iters, 1) as iv:
                    body(iv)

    nc.compile()
    _NC_CACHE[iters] = nc
    return nc


def _prep_core_inputs(x, mask_w, mask_b, filt_w, filt_b):
    """Host-side layout prep. Returns (shared replicated dict, per-b x maps)."""
    bf = ml_dtypes.bfloat16
    fw_kg = CFG["fw_kg"]
    mask_wT = np.ascontiguousarray(mask_w.T).reshape(CJ, P, K).astype(bf)
    mb = mask_b.reshape(K, 1).astype(np.float32)
    # fw[g, p, kk*2048 + j*512 + d] = filt_w[kg*g+kk, d, 128j+p]
    fwT = filt_w.transpose(0, 2, 1).astype(bf)          # [k, c', d]
    fw = fwT.reshape(K // fw_kg, fw_kg, CJ, P, 512).transpose(0, 3, 1, 2, 4)
    fw = np.ascontiguousarray(fw).reshape(K // fw_kg, P, fw_kg * CJ * 512)
    fb = filt_b.astype(np.float32)

    shared = {"mask_wT": mask_wT, "mask_b": mb, "fw": fw, "filt_b": fb}

    in_maps = []
    for b in range(B):
        xb = x[b].reshape(C, HW).astype(bf)             # [512, 16384]
        x_nat = xb.reshape(CJ, P, HW)
        # xT[g, p, ii*2048 + t*512 + c] = x[c, 512*(2g+ii) + 128t + p]
        xT = xb.reshape(C, NCH, 4, P).transpose(1, 3, 2, 0)
        xT = np.ascontiguousarray(xT).reshape(NCH // 2, 2, P, 2048)
        xT = np.ascontiguousarray(xT.transpose(0, 2, 1, 3)).reshape(
            NCH // 2, P, 4096)
        in_maps.append({"x_nat": x_nat, "xT": xT, **shared})
    return in_maps


def kernel(x, mask_w, mask_b, filt_w, filt_b):
    x = np.asarray(x, dtype=np.float32)
    mask_w = np.asarray(mask_w, dtype=np.float32)
    mask_b = np.asarray(mask_b, dtype=np.float32)
    filt_w = np.asarray(filt_w, dtype=np.float32)
    filt_b = np.asarray(filt_b, dtype=np.float32)

    nc = build_nc(iters=1)
    in_maps = _prep_core_inputs(x, mask_w, mask_b, filt_w, filt_b)
    res = run_bass_kernel_spmd(nc, in_maps, list(range(N_CORES)))

    pred = np.stack([res.results[b]["pred"].reshape(K, H, W) for b in range(B)])
    aux = np.stack([res.results[b]["aux"].reshape(K, H, W) for b in range(B)])
    return (pred.astype(np.float32), aux.astype(np.float32))



# revision 7
# speedup vs baseline: 3.5641x; 3.5641x over previous
"""Trainium2 Bass kernel for ConditionalFilterLayer.

Reference computation (per sample b):
  aux   = sigmoid(mask_w @ x + mask_b)          [K, HW]
  cf    = (aux @ x.T) / HW                      [K, C]
  filt  = batched_k(filt_w[k] @ cf[k]) + filt_b [K, C]
  pred  = filt @ x                              [K, HW]

Sharding: data-parallel over batch for phases A (mask conv + pooling) and
C (dynamic conv): B=8 == 8 cores, one sample per core. Phase B (filter
generation) is sharded over K: core m holds filt_w[8m:8m+8] only (4.2 MB
bf16 instead of a replicated 33.5 MB), with two tiny AllToAll collectives
moving cf (to k-owners) and filt (back to batch-owners), ~64 KB each.

Per-core layout choices:
  x_nat  bf16 [4, 128, 16384]  c-major, resident in SBUF (128 KB/partition)
  xT8    f8e3 [16, 128, 4096]  hw-major chunks for the pooling contraction
  fw8    bf16 [8, 4, 128, 512] own k-shard of filt_w.T (pre-scaled by 1/HW)
Logit and pred matmuls are column-tiled: two hw-chunks' K=64 outputs run
concurrently in PE column groups (0,0)/(0,64), sharing one PSUM bank via
per-element has_written (only the first matmul in a bank uses start=True).
Pooling contracts PE-transposed aux (fp8e3) against xT8 (fp8e3).
Outputs are written bf16 in stacked-chunk-pair layout and fixed on host.
"""
import sys

if "/opt/trn_rl_repo" not in sys.path:
    sys.path.insert(0, "/opt/trn_rl_repo")

import numpy as np
import ml_dtypes

import concourse.bass as bass
import concourse.mybir as mybir
import concourse.tile as tile
from concourse import bacc
from concourse.bass_utils import run_bass_kernel_spmd
from concourse.masks import make_identity

BF16 = mybir.dt.bfloat16
F8 = mybir.dt.float8e3
F32 = mybir.dt.float32

B, C, K, H, W = 8, 512, 64, 128, 128
HW = H * W            # 16384
P = 128
CJ = C // P           # 4 contraction chunks
NCH = HW // 512       # 32 hw chunks of 512
NPAIR = NCH // 2      # 16 chunk pairs
KSH = K // 8          # 8 k's per core
N_CORES = 8

_NC_CACHE = {}

CFG = dict(
    psA_bufs=3,
    psT_bufs=2,
    xt_bufs=3,
    aux_bufs=2,
    auxT_bufs=2,
    pred_bufs=2,
    out_grp=2,        # chunk-pairs per output DMA
)


def build_nc(iters: int = 1, **over):
    cfg = {**CFG, **over}
    key = (iters, tuple(sorted(cfg.items())))
    if key in _NC_CACHE:
        return _NC_CACHE[key]
    out_grp = cfg["out_grp"]

    nc = bacc.Bacc("TRN2", target_bir_lowering=False, debug=False)

    x_dram = nc.dram_tensor("x_nat", [CJ, P, HW], BF16, kind="ExternalInput")
    xt_dram = nc.dram_tensor("xT8", [NPAIR, P, 4096], F8, kind="ExternalInput")
    mw_dram = nc.dram_tensor("mask_wT", [CJ, P, K], BF16, kind="ExternalInput")
    mb_dram = nc.dram_tensor("mask_b2", [P, 1], F32, kind="ExternalInput")
    fw_dram = nc.dram_tensor("fw8", [KSH, CJ, P, 512], BF16, kind="ExternalInput")
    fb_dram = nc.dram_tensor("fb_pad", [2, P, 512], F32, kind="ExternalInput")
    aux_dram = nc.dram_tensor("aux_s", [P, NPAIR, 512], BF16, kind="ExternalOutput")
    pred_dram = nc.dram_tensor("pred_s", [P, NPAIR, 512], BF16, kind="ExternalOutput")

    with tile.TileContext(nc) as tc:
        with (
            tc.tile_pool(name="const", bufs=1) as constp,
            tc.tile_pool(name="xres", bufs=1) as xresp,
            tc.tile_pool(name="fwres", bufs=1) as fwresp,
            tc.tile_pool(name="xt", bufs=cfg["xt_bufs"]) as xtp,
            tc.tile_pool(name="aux", bufs=cfg["aux_bufs"]) as auxp,
            tc.tile_pool(name="auxT", bufs=cfg["auxT_bufs"]) as auxTp,
            tc.tile_pool(name="small", bufs=1) as smallp,
            tc.tile_pool(name="pred", bufs=cfg["pred_bufs"]) as predp,
            tc.tile_pool(name="psA", bufs=cfg["psA_bufs"], space="PSUM") as psA,
            tc.tile_pool(name="psT", bufs=cfg["psT_bufs"], space="PSUM") as psT,
            tc.tile_pool(name="psCF", bufs=1, space="PSUM") as psCF,
            tc.tile_pool(name="dram", bufs=2, space="DRAM") as dramp,
        ):
            # ---- constants (outside the timed body) ----
            ident = constp.tile([P, P], BF16)
            make_identity(nc, ident[:])
            mw_sb = constp.tile([P, CJ, K], BF16)
            nc.sync.dma_start(mw_sb[:], mw_dram.rearrange("j p k -> p j k"))
            mb_sb = constp.tile([P, 1], F32)
            nc.sync.dma_start(mb_sb[:], mb_dram[:])
            fb_sb = constp.tile([P, 2, 512], F32)
            nc.sync.dma_start(fb_sb[:], fb_dram.rearrange("t p d -> p t d"))

            def body(it):
                # ---------- resident loads ----------
                x_res = xresp.tile([P, CJ, HW], BF16)
                for h in range(4):
                    for j in range(CJ):
                        nc.sync.dma_start(
                            x_res[:, j, h * 4096:(h + 1) * 4096],
                            x_dram[j, :, h * 4096:(h + 1) * 4096],
                        )
                fw_res = fwresp.tile([P, KSH, CJ, 512], BF16)
                nc.sync.dma_start(fw_res[:], fw_dram.rearrange("k j p d -> p k j d"))

                # ---------- phase A: logits+sigmoid (col-tiled pairs), pooling
                ps_cf = psCF.tile([K, 512], F32)
                for g in range(NPAIR):
                    xt_t = xtp.tile([P, 4096], F8)
                    nc.scalar.dma_start(xt_t[:], xt_dram[g])

                    ps_l = psA.tile([P, 512], F32, tag="psA")
                    c0, c1 = 2 * g, 2 * g + 1
                    for j in range(CJ):
                        nc.tensor.matmul(
                            ps_l[0:64, :], mw_sb[:, j, :],
                            x_res[:, j, c0 * 512:(c0 + 1) * 512],
                            start=(j == 0), stop=(j == CJ - 1),
                        )
                        nc.tensor.matmul(
                            ps_l[64:128, :], mw_sb[:, j, :],
                            x_res[:, j, c1 * 512:(c1 + 1) * 512],
                            start=(j == 0), stop=(j == CJ - 1),
                        )
                    if g % out_grp == 0:
                        aux_g = auxp.tile([P, out_grp, 512], BF16)
                    aux_st = aux_g[:, g % out_grp, :]
                    nc.scalar.activation(
                        aux_st, ps_l[:],
                        mybir.ActivationFunctionType.Sigmoid,
                        bias=mb_sb[:, 0:1],
                    )
                    if g % out_grp == out_grp - 1:
                        nc.sync.dma_start(
                            aux_dram[:, g - out_grp + 1:g + 1, :], aux_g[:],
                        )

                    ps_tr = psT.tile([P, 4, P], BF16, tag="psT")
                    for t in range(4):
                        nc.tensor.transpose(
                            ps_tr[:, t, :], aux_st[:, t * 128:(t + 1) * 128],
                            ident[:],
                        )
                    auxT_t = auxTp.tile([P, 4, P], F8)
                    nc.vector.tensor_copy(auxT_t[:], ps_tr[:])
                    for t in range(4):
                        nc.tensor.matmul(
                            ps_cf[:], auxT_t[:, t, 0:64],
                            xt_t[:, t * 512:(t + 1) * 512],
                            start=(g == 0 and t == 0), stop=False,
                        )
                        nc.tensor.matmul(
                            ps_cf[:], auxT_t[:, t, 64:128],
                            xt_t[:, 2048 + t * 512:2048 + (t + 1) * 512],
                            start=False,
                            stop=(g == NPAIR - 1 and t == 3),
                        )

                # ---------- phase B: cf A2A -> k-sharded filters -> filt A2A
                cf_bf = smallp.tile([K, 512], BF16, tag="cf_bf")
                nc.scalar.activation(
                    cf_bf[:], ps_cf[:], mybir.ActivationFunctionType.Copy,
                )
                cf_in = dramp.tile([K, 512], BF16)
                cf_out = dramp.tile([K, 512], BF16)
                nc.gpsimd.dma_start(cf_in[:], cf_bf[:])
                nc.gpsimd.collective_compute(
                    "AllToAll", mybir.AluOpType.bypass,
                    replica_groups=[list(range(N_CORES))],
                    ins=[cf_in.opt()], outs=[cf_out.opt()],
                )
                cf_recv = smallp.tile([K, 512], BF16, tag="cf_recv")
                nc.gpsimd.dma_start(cf_recv[:], cf_out[:])

                ps_tr = psT.tile([P, 4, P], BF16, tag="psT")
                for j in range(CJ):
                    nc.tensor.transpose(
                        ps_tr[:, j, 0:64], cf_recv[:, j * 128:(j + 1) * 128],
                        ident[0:64, 0:64],
                    )
                cfT = smallp.tile([P, CJ, K], BF16, tag="cfT")
                nc.vector.tensor_copy(cfT[:], ps_tr[:, :, 0:64])

                fi_in = dramp.tile([8, 8, 512], BF16)   # [b_peer, kk, d]
                fi_out = dramp.tile([K, 512], BF16)     # rows = k (=8*peer+kk)
                for half in range(2):
                    ps_f = psA.tile([P, 512], F32, tag="psA")
                    for j in range(CJ):
                        for q in range(4):
                            kk = 4 * half + q
                            nc.tensor.matmul(
                                ps_f[32 * q:32 * q + 8, :],
                                cfT[:, j, kk:64:8],
                                fw_res[:, kk, j, :],
                                start=(j == 0),
                                stop=(j == CJ - 1),
                                tile_position=(0, 32 * q),
                            )
                    f_tmp = smallp.tile([P, 512], BF16, tag=f"ftmp{half}")
                    nc.vector.tensor_add(f_tmp[:], ps_f[:], fb_sb[:, half, :])
                    for q in range(4):
                        nc.sync.dma_start(
                            fi_in[:, 4 * half + q, :],
                            f_tmp[32 * q:32 * q + 8, :],
                        )
                nc.gpsimd.collective_compute(
                    "AllToAll", mybir.AluOpType.bypass,
                    replica_groups=[list(range(N_CORES))],
                    ins=[fi_in.opt()], outs=[fi_out.opt()],
                )
                filt_recv = smallp.tile([K, 512], BF16, tag="filt_recv")
                nc.gpsimd.dma_start(filt_recv[:], fi_out[:])

                ps_tr = psT.tile([P, 4, P], BF16, tag="psT")
                for j in range(CJ):
                    nc.tensor.transpose(
                        ps_tr[:, j, 0:64], filt_recv[:, j * 128:(j + 1) * 128],
                        ident[0:64, 0:64],
                    )
                filtT = smallp.tile([P, CJ, K], BF16, tag="filtT")
                nc.vector.tensor_copy(filtT[:], ps_tr[:, :, 0:64])

                # ---------- phase C: pred = filt @ x (col-tiled pairs)
                for g in range(NPAIR):
                    ps_p = psA.tile([P, 512], F32, tag="psA")
                    c0, c1 = 2 * g, 2 * g + 1
                    for j in range(CJ):
                        nc.tensor.matmul(
                            ps_p[0:64, :], filtT[:, j, :],
                            x_res[:, j, c0 * 512:(c0 + 1) * 512],
                            start=(j == 0), stop=(j == CJ - 1),
                        )
                        nc.tensor.matmul(
                            ps_p[64:128, :], filtT[:, j, :],
                            x_res[:, j, c1 * 512:(c1 + 1) * 512],
                            start=(j == 0), stop=(j == CJ - 1),
                        )
                    if g % out_grp == 0:
                        pred_g = predp.tile([P, out_grp, 512], BF16)
                    nc.vector.tensor_copy(pred_g[:, g % out_grp, :], ps_p[:])
                    if g % out_grp == out_grp - 1:
                        nc.sync.dma_start(
                            pred_dram[:, g - out_grp + 1:g + 1, :], pred_g[:],
                        )

            for it in range(iters):
                body(it)

    nc.compile()
    _NC_CACHE[key] = nc
    return nc


def _prep_core_inputs(x, mask_w, mask_b, filt_w, filt_b):
    """Host-side layout prep. Returns per-core input dicts."""
    bf = ml_dtypes.bfloat16
    f8 = ml_dtypes.float8_e3m4
    mask_wT = np.ascontiguousarray(mask_w.T).reshape(CJ, P, K).astype(bf)
    mb2 = np.concatenate([mask_b, mask_b]).reshape(P, 1).astype(np.float32)

    # fw8[core][kk, j, p, d] = filt_w[8m+kk, d, 128j+p] / HW
    fws = (filt_w.transpose(0, 2, 1) / HW).astype(bf)     # [k, c, d]
    fw_all = fws.reshape(B, KSH, CJ, P, 512)              # [m, kk, j, p, d]

    fb_all = np.zeros((B, 2, P, 512), np.float32)
    for m in range(B):
        for half in range(2):
            for q in range(4):
                kk = 4 * half + q
                fb_all[m, half, 32 * q:32 * q + 8, :] = filt_b[8 * m + kk][None, :]

    in_maps = []
    for b in range(B):
        xb = x[b].reshape(C, HW)
        x_nat = xb.astype(bf).reshape(CJ, P, HW)
        # xT8[g, p, ii*2048 + t*512 + c] = x[c, (2g+ii)*512 + 128t + p]
        t1 = xb.reshape(C, NPAIR, 2, 4, P).transpose(1, 4, 2, 3, 0)
        xT8 = np.ascontiguousarray(t1).reshape(NPAIR, P, 4096).astype(f8)
        in_maps.append({
            "x_nat": x_nat, "xT8": xT8, "mask_wT": mask_wT, "mask_b2": mb2,
            "fw8": fw_all[b], "fb_pad": fb_all[b],
        })
    return in_maps


def _unstack(a_s):
    """[128, NPAIR, 512] stacked-chunk-pair layout -> [K, H, W] float32."""
    a = np.asarray(a_s, dtype=np.float32)
    out = np.empty((K, NPAIR, 2, 512), np.float32)
    out[:, :, 0, :] = a[0:64]
    out[:, :, 1, :] = a[64:128]
    return out.reshape(K, H, W)


def kernel(x, mask_w, mask_b, filt_w, filt_b):
    x = np.asarray(x, dtype=np.float32)
    mask_w = np.asarray(mask_w, dtype=np.float32)
    mask_b = np.asarray(mask_b, dtype=np.float32)
    filt_w = np.asarray(filt_w, dtype=np.float32)
    filt_b = np.asarray(filt_b, dtype=np.float32)

    nc = build_nc(iters=1)
    in_maps = _prep_core_inputs(x, mask_w, mask_b, filt_w, filt_b)
    res = run_bass_kernel_spmd(nc, in_maps, list(range(N_CORES)))

    pred = np.stack([_unstack(res.results[b]["pred_s"]) for b in range(B)])
    aux = np.stack([_unstack(res.results[b]["aux_s"]) for b in range(B)])
    return (pred, aux)
